# revision 8
# baseline (speedup 1.0000x reference)
"""Self-contained Trainium2 Bass kernel for the 2-layer GCN (nn_Net_21646635172356).

kernel(**inputs) takes FULL inputs (x [100000,512] f32, W1 [512,16], b1 [16],
W2 [16,7], b2 [7], edge_index [2,3200000] int64) and returns the FULL
log-softmax output [100000, 7] f32.

Design (8 NeuronCores, SPMD):
- Nodes globally sorted by degree (desc), strided across cores; each core owns
  12544 node slots (98 tiles x 128 partitions; 12500 real + 44 zero pads).
- norm factorizes: norm_e = dinv[src]*dinv[dst]; rows pre-scaled by dinv before
  the all-gather, aggregates post-scaled by dinv[dst]. Self-loops added densely.
- h' = dinv * (x @ W1) computed per-shard on TensorE (x shipped pre-transposed),
  all-gathered into a DRAM table [100352, 16]; layer 2 likewise [100352, 8].
- Aggregation: host-built padded CSR (per 7-tile group width W_g, zero-row
  padding, ~7% inflation). Per (tile, w): one indirect DMA gathers 128 rows
  (one per dst partition) and CCE-accumulates into the tile's SBUF agg buffer.
- relu/bias/matmul W2/log_softmax on device; host only re-permutes rows.
"""

import os
import sys
import numpy as np

sys.path.insert(0, "/opt/trn_rl_repo")

N_NODES = 100000
N_EDGES = 3200000
IN_F, HID, N_CLS = 512, 16, 7
N_CORES = 8
P = 128
N_TILES = 98
TILE_GROUP = 7
N_GROUPS = N_TILES // TILE_GROUP  # 14
N_LOC = N_TILES * P               # 12544
HID2 = 8
TROWS = N_CORES * N_LOC           # 100352
PAD_I = N_LOC - 1                 # zero-row sorted-position per core


def _l_of_i(i):
    t, p = i // P, i % P
    return p * N_TILES + t


def preprocess(x, W1, b1, W2, b2, edge_index):
    x = np.asarray(x, dtype=np.float32)
    src = np.asarray(edge_index[0], dtype=np.int64)
    dst = np.asarray(edge_index[1], dtype=np.int64)
    n_real = N_NODES // N_CORES  # 12500

    deg = np.bincount(dst, minlength=N_NODES).astype(np.int64) + 1
    order = np.argsort(-deg, kind="stable")
    rank_of = np.empty(N_NODES, dtype=np.int64)
    rank_of[order] = np.arange(N_NODES)
    core_of = rank_of % N_CORES
    pos_of = rank_of // N_CORES
    trow_of = core_of * N_LOC + _l_of_i(pos_of)

    xT = np.zeros((N_CORES, IN_F, N_LOC), dtype=np.float32)
    degt = np.ones((N_CORES, P, N_TILES), dtype=np.float32)
    for k in range(N_CORES):
        nodes_k = order[k::N_CORES]
        i = np.arange(len(nodes_k))
        xT[k, :, i] = x[nodes_k]
        degt[k, i % P, i // P] = deg[nodes_k]

    # group widths (same across cores; excludes self loop -> in-deg only)
    W = np.zeros(N_GROUPS, dtype=np.int64)
    indeg = deg - 1
    for g in range(N_GROUPS):
        lo, hi = g * TILE_GROUP * P, (g + 1) * TILE_GROUP * P
        sel = (pos_of >= lo) & (pos_of < hi)
        W[g] = max(1, indeg[sel].max())

    # idx blobs: per core, per group: [W_g, TILE_GROUP, P] int32 table rows
    # (slot (w, tt, p) = w-th in-edge of dst node i=(g*TG+tt)*P+p), padded
    # with the core-local zero row.
    e_core = core_of[dst]
    idx_blobs = []
    for k in range(N_CORES):
        pad_row = k * N_LOC + _l_of_i(PAD_I)
        m = e_core == k
        s_k, d_k = src[m], dst[m]
        pos_d = pos_of[d_k]
        o = np.argsort(pos_d, kind="stable")
        s_k, pos_d = s_k[o], pos_d[o]
        bounds = np.searchsorted(pos_d, np.arange(n_real + 1))
        blobs = []
        for g in range(N_GROUPS):
            Wg = int(W[g])
            blob = np.full((Wg, TILE_GROUP, P), pad_row, dtype=np.int32)
            base = g * TILE_GROUP * P
            for tt in range(TILE_GROUP):
                for p in range(P):
                    i = base + tt * P + p
                    if i >= n_real:
                        continue
                    lo, hi = bounds[i], bounds[i + 1]
                    blob[: hi - lo, tt, p] = trow_of[s_k[lo:hi]]
            blobs.append(blob.reshape(Wg * TILE_GROUP, P))
        idx_blobs.append(np.ascontiguousarray(np.concatenate(blobs, axis=0).T))  # [P, sum(Wg)*TG]

    W2p = np.zeros((HID, HID2), dtype=np.float32)
    W2p[:, :N_CLS] = np.asarray(W2, dtype=np.float32)
    b1t = np.broadcast_to(np.asarray(b1, np.float32), (P, HID)).copy()
    b2t = np.zeros((P, HID2), dtype=np.float32)
    b2t[:, :N_CLS] = np.asarray(b2, np.float32)

    return dict(xT=xT, degt=degt, idx=np.stack(idx_blobs), W=W,
                W1=np.asarray(W1, np.float32), W2p=W2p, b1t=b1t, b2t=b2t,
                order=order)


_BUILD_CACHE = {}


def build(W):
    """Build + compile the SPMD bass program for group widths W (tuple)."""
    key = tuple(int(w) for w in W)
    if key in _BUILD_CACHE:
        return _BUILD_CACHE[key]

    from concourse import bass, mybir, bacc
    import concourse.tile as tile
    from concourse.masks import make_identity

    IDX_ROWS = int(sum(W)) * TILE_GROUP  # rows of [P] int32 indices

    nc = bacc.Bacc("TRN2", target_bir_lowering=False, debug=False,
                   num_devices=N_CORES)
    f32, i32 = mybir.dt.float32, mybir.dt.int32

    xT_in = nc.dram_tensor("xT", [IN_F, N_LOC], f32, kind="ExternalInput")
    degt_in = nc.dram_tensor("degt", [P, N_TILES], f32, kind="ExternalInput")
    idx_in = nc.dram_tensor("idx", [P, IDX_ROWS], i32, kind="ExternalInput")
    W1_in = nc.dram_tensor("W1", [IN_F, HID], f32, kind="ExternalInput")
    W2_in = nc.dram_tensor("W2p", [HID, HID2], f32, kind="ExternalInput")
    b1_in = nc.dram_tensor("b1t", [P, HID], f32, kind="ExternalInput")
    b2_in = nc.dram_tensor("b2t", [P, HID2], f32, kind="ExternalInput")
    out_ext = nc.dram_tensor("out", [N_LOC, N_CLS], f32, kind="ExternalOutput")

    shard1 = nc.dram_tensor("shard1", [N_LOC, HID], f32)
    table1 = nc.dram_tensor("table1", [TROWS, HID], f32, addr_space="Shared")
    shard2 = nc.dram_tensor("shard2", [N_LOC, HID2], f32)
    table2 = nc.dram_tensor("table2", [TROWS, HID2], f32, addr_space="Shared")

    with tile.TileContext(nc) as tc:
        with (
            tc.tile_pool(name="const", bufs=1) as cpool,
            tc.tile_pool(name="xts", bufs=2) as xpool,
            tc.tile_pool(name="idxp", bufs=1) as ipool,
            tc.tile_pool(name="work", bufs=1) as wpool,
            tc.tile_pool(name="psum", bufs=2, space="PSUM") as psum,
            tc.tile_pool(name="psum2", bufs=2, space="PSUM") as psum2,
        ):
            w1 = cpool.tile([P, 4 * HID], f32)  # 4 k-chunks side by side
            for kc in range(4):
                nc.sync.dma_start(out=w1[:, kc * HID:(kc + 1) * HID],
                                  in_=W1_in[kc * P:(kc + 1) * P, :])
            w2 = cpool.tile([HID, HID2], f32)
            nc.sync.dma_start(out=w2[:], in_=W2_in[:, :])
            b1 = cpool.tile([P, HID], f32)
            nc.sync.dma_start(out=b1[:], in_=b1_in[:, :])
            b2 = cpool.tile([P, HID2], f32)
            nc.sync.dma_start(out=b2[:], in_=b2_in[:, :])
            degt = cpool.tile([P, N_TILES], f32)
            nc.sync.dma_start(out=degt[:], in_=degt_in[:, :])
            dinv = cpool.tile([P, N_TILES], f32)
            nc.scalar.activation(dinv[:], degt[:],
                                 mybir.ActivationFunctionType.Sqrt)
            nc.vector.reciprocal(dinv[:], dinv[:])
            ident = cpool.tile([P, P], f32)
            make_identity(nc, ident[:])

            # whole idx blob resident in SBUF: [P, IDX_ROWS] int32
            idxs = ipool.tile([P, IDX_ROWS], i32)
            nc.sync.dma_start(out=idxs[:], in_=idx_in[:, :])

            hp = wpool.tile([P, N_TILES * HID], f32)   # h' rows (pre-scaled)
            # ---- Phase A: h' = dinv * (x @ W1), node tiles of 128 ----
            CH = TILE_GROUP * P  # 896 node-columns per chunk
            for g in range(N_GROUPS):
                xt = []
                for kc in range(4):
                    xtk = xpool.tile([P, CH], f32, tag=f"xt{kc}")
                    xt.append(xtk)
                for kc in range(4):
                    nc.sync.dma_start(
                        out=xt[kc][:],
                        in_=xT_in[kc * P:(kc + 1) * P, g * CH:(g + 1) * CH])
                for tt in range(TILE_GROUP):
                    t = g * TILE_GROUP + tt
                    ph = psum.tile([P, HID], f32, tag="ph")
                    for kc in range(4):
                        nc.tensor.matmul(
                            out=ph[:],
                            lhsT=xt[kc][:, tt * P:(tt + 1) * P],
                            rhs=w1[:, kc * HID:(kc + 1) * HID],
                            start=(kc == 0), stop=(kc == 3))
                    nc.vector.tensor_tensor(
                        out=hp[:, t * HID:(t + 1) * HID],
                        in0=ph[:],
                        in1=dinv[:, t:t + 1].to_broadcast([P, HID]),
                        op=mybir.AluOpType.mult)

            nc.sync.dma_start(
                out=shard1[:, :].rearrange("(p t) f -> p (t f)", p=P),
                in_=hp[:])
            nc.gpsimd.collective_compute(
                "AllGather", mybir.AluOpType.bypass,
                replica_groups=[list(range(N_CORES))],
                ins=[shard1[:, :]], outs=[table1[:, :]])

            # ---- Phase B: L1 aggregation via gather-accumulate ----
            agg1 = wpool.tile([P, N_TILES * HID], f32)
            # init agg = h' (self-loop contribution; rows already dinv-scaled)
            nc.vector.tensor_copy(out=agg1[:], in_=hp[:])

            roff = 0
            for g in range(N_GROUPS):
                for w in range(int(W[g])):
                    for tt in range(TILE_GROUP):
                        t = g * TILE_GROUP + tt
                        nc.gpsimd.indirect_dma_start(
                            out=agg1[:, t * HID:(t + 1) * HID],
                            out_offset=None,
                            in_=table1[:, :],
                            in_offset=bass.IndirectOffsetOnAxis(
                                ap=idxs[:, roff:roff + 1], axis=0),
                            compute_op=mybir.AluOpType.add)
                        roff += 1

            # h1 = relu(dinv*agg1 + b1)  (in place on agg1)
            for t in range(N_TILES):
                nc.vector.tensor_tensor(
                    out=agg1[:, t * HID:(t + 1) * HID],
                    in0=agg1[:, t * HID:(t + 1) * HID],
                    in1=dinv[:, t:t + 1].to_broadcast([P, HID]),
                    op=mybir.AluOpType.mult)
            a3 = agg1[:].rearrange("p (t f) -> p t f", f=HID)
            nc.vector.tensor_tensor(
                out=a3, in0=a3,
                in1=b1[:].rearrange("p (o f) -> p o f", o=1).to_broadcast(
                    [P, N_TILES, HID]),
                op=mybir.AluOpType.add)
            nc.scalar.activation(agg1[:], agg1[:],
                                 mybir.ActivationFunctionType.Relu)

            # ---- Phase C: h2' = dinv * (h1 @ W2p) ----
            h2p = wpool.tile([P, N_TILES * HID2], f32)
            for t in range(N_TILES):
                pt = psum.tile([HID, P], f32, tag="pt")
                nc.tensor.transpose(out=pt[:], in_=agg1[:, t * HID:(t + 1) * HID],
                                    identity=ident[:])
                h1T = wpool.tile([HID, P], f32, tag="h1T")
                nc.vector.tensor_copy(out=h1T[:], in_=pt[:])
                p2 = psum2.tile([P, HID2], f32, tag="p2")
                nc.tensor.matmul(out=p2[:], lhsT=h1T[:], rhs=w2[:],
                                 start=True, stop=True)
                nc.vector.tensor_tensor(
                    out=h2p[:, t * HID2:(t + 1) * HID2],
                    in0=p2[:],
                    in1=dinv[:, t:t + 1].to_broadcast([P, HID2]),
                    op=mybir.AluOpType.mult)

            nc.sync.dma_start(
                out=shard2[:, :].rearrange("(p t) f -> p (t f)", p=P),
                in_=h2p[:])
            nc.gpsimd.collective_compute(
                "AllGather", mybir.AluOpType.bypass,
                replica_groups=[list(range(N_CORES))],
                ins=[shard2[:, :]], outs=[table2[:, :]])

            # ---- Phase D: L2 aggregation ----
            agg2 = wpool.tile([P, N_TILES * HID2], f32)
            nc.vector.tensor_copy(out=agg2[:], in_=h2p[:])
            roff = 0
            for g in range(N_GROUPS):
                for w in range(int(W[g])):
                    for tt in range(TILE_GROUP):
                        t = g * TILE_GROUP + tt
                        nc.gpsimd.indirect_dma_start(
                            out=agg2[:, t * HID2:(t + 1) * HID2],
                            out_offset=None,
                            in_=table2[:, :],
                            in_offset=bass.IndirectOffsetOnAxis(
                                ap=idxs[:, roff:roff + 1], axis=0),
                            compute_op=mybir.AluOpType.add)
                        roff += 1

            # ---- Phase E: z = dinv*agg2 + b2; log_softmax over 7 cols ----
            for t in range(N_TILES):
                nc.vector.tensor_tensor(
                    out=agg2[:, t * HID2:(t + 1) * HID2],
                    in0=agg2[:, t * HID2:(t + 1) * HID2],
                    in1=dinv[:, t:t + 1].to_broadcast([P, HID2]),
                    op=mybir.AluOpType.mult)
            g3 = agg2[:].rearrange("p (t f) -> p t f", f=HID2)
            nc.vector.tensor_tensor(
                out=g3, in0=g3,
                in1=b2[:].rearrange("p (o f) -> p o f", o=1).to_broadcast(
                    [P, N_TILES, HID2]),
                op=mybir.AluOpType.add)

            z = agg2[:].rearrange("p (t f) -> p t f", f=HID2)
            z7 = z[:, :, 0:N_CLS]
            m = wpool.tile([P, N_TILES], f32)
            nc.vector.tensor_reduce(out=m[:], in_=z7, axis=mybir.AxisListType.X,
                                    op=mybir.AluOpType.max)
            zs = wpool.tile([P, N_TILES * N_CLS], f32)
            zs3 = zs[:].rearrange("p (t f) -> p t f", f=N_CLS)
            nc.vector.tensor_tensor(
                out=zs3, in0=z7,
                in1=m[:].rearrange("p (t o) -> p t o", o=1).to_broadcast(
                    [P, N_TILES, N_CLS]),
                op=mybir.AluOpType.subtract)
            e = wpool.tile([P, N_TILES * N_CLS], f32)
            nc.scalar.activation(e[:], zs[:], mybir.ActivationFunctionType.Exp)
            s = wpool.tile([P, N_TILES], f32)
            nc.vector.tensor_reduce(
                out=s[:], in_=e[:].rearrange("p (t f) -> p t f", f=N_CLS),
                axis=mybir.AxisListType.X, op=mybir.AluOpType.add)
            ls = wpool.tile([P, N_TILES], f32)
            nc.scalar.activation(ls[:], s[:], mybir.ActivationFunctionType.Ln)
            nc.vector.tensor_tensor(
                out=zs3, in0=zs3,
                in1=ls[:].rearrange("p (t o) -> p t o", o=1).to_broadcast(
                    [P, N_TILES, N_CLS]),
                op=mybir.AluOpType.subtract)

            nc.sync.dma_start(
                out=out_ext[:, :].rearrange("(p t) f -> p (t f)", p=P),
                in_=zs[:])
    nc.compile()
    _BUILD_CACHE[key] = nc
    return nc


def kernel(x, W1, b1, W2, b2, edge_index):
    from concourse.bass_utils import run_bass_kernel_spmd

    prep = preprocess(x, W1, b1, W2, b2, edge_index)
    nc = build(prep["W"])

    in_maps = []
    for k in range(N_CORES):
        in_maps.append({
            "xT": prep["xT"][k],
            "degt": prep["degt"][k],
            "idx": prep["idx"][k],
            "W1": prep["W1"],
            "W2p": prep["W2p"],
            "b1t": prep["b1t"],
            "b2t": prep["b2t"],
        })

    global LAST_EXEC_NS
    if os.environ.get("GCN_SIM", "") == "1":
        from concourse import bass_interp
        sim = bass_interp.MultiCoreSim(nc, N_CORES)
        for k in range(N_CORES):
            for name, arr in in_maps[k].items():
                sim.cores[k].tensor(name)[:] = arr
        sim.simulate()
        class _R: pass
        res = _R()
        res.results = [{"out": np.array(sim.cores[k].tensor("out"))}
                       for k in range(N_CORES)]
        LAST_EXEC_NS = int(sim.global_time)
    else:
        trace = os.environ.get("GCN_TRACE", "") == "1"
        try:
            res = run_bass_kernel_spmd(nc, in_maps,
                                       core_ids=list(range(N_CORES)),
                                       trace=trace)
        except ModuleNotFoundError:
            res = run_bass_kernel_spmd(nc, in_maps,
                                       core_ids=list(range(N_CORES)))
        LAST_EXEC_NS = res.exec_time_ns

    order = prep["order"]
    out = np.zeros((N_NODES, N_CLS), dtype=np.float32)
    i = np.arange(N_NODES // N_CORES)
    li = _l_of_i(i)
    for k in range(N_CORES):
        out[order[k::N_CORES]] = res.results[k]["out"][li]
    return out


LAST_EXEC_NS = None


# revision 9
# speedup vs baseline: 1.0393x; 1.0393x over previous
"""Self-contained Trainium2 Bass kernel for the 2-layer GCN (nn_Net_21646635172356).

kernel(**inputs) takes FULL inputs (x [100000,512] f32, W1 [512,16], b1 [16],
W2 [16,7], b2 [7], edge_index [2,3200000] int64) and returns the FULL
log-softmax output [100000, 7] f32.

Design (8 NeuronCores, SPMD):
- Nodes globally sorted by degree (desc), strided across cores; each core owns
  12544 node slots (98 tiles x 128 partitions; 12500 real + 44 zero pads).
- norm factorizes: norm_e = dinv[src]*dinv[dst]; rows pre-scaled by dinv before
  the all-gather, aggregates post-scaled by dinv[dst]. Self-loops added densely.
- h' = dinv * (x @ W1) computed per-shard on TensorE (x shipped pre-transposed),
  all-gathered into a DRAM table [100352, 16]; layer 2 likewise [100352, 8].
- Aggregation: host-built padded CSR (per 7-tile group width W_g, zero-row
  padding, ~7% inflation). Per (tile, w): one indirect DMA gathers 128 rows
  (one per dst partition) and CCE-accumulates into the tile's SBUF agg buffer.
- relu/bias/matmul W2/log_softmax on device; host only re-permutes rows.
"""

import os
import sys
import numpy as np

sys.path.insert(0, "/opt/trn_rl_repo")

N_NODES = 100000
N_EDGES = 3200000
IN_F, HID, N_CLS = 512, 16, 7
N_CORES = 8
P = 128
N_TILES = 98
TILE_GROUP = 7
N_GROUPS = N_TILES // TILE_GROUP  # 14
N_LOC = N_TILES * P               # 12544
HID2 = 8
TROWS = N_CORES * N_LOC           # 100352
PAD_I = N_LOC - 1                 # zero-row sorted-position per core


def _l_of_i(i):
    t, p = i // P, i % P
    return p * N_TILES + t


def preprocess(x, W1, b1, W2, b2, edge_index):
    x = np.asarray(x, dtype=np.float32)
    src = np.asarray(edge_index[0], dtype=np.int64)
    dst = np.asarray(edge_index[1], dtype=np.int64)
    n_real = N_NODES // N_CORES  # 12500

    deg = np.bincount(dst, minlength=N_NODES).astype(np.int64) + 1
    order = np.argsort(-deg, kind="stable")
    rank_of = np.empty(N_NODES, dtype=np.int64)
    rank_of[order] = np.arange(N_NODES)
    core_of = rank_of % N_CORES
    pos_of = rank_of // N_CORES
    trow_of = core_of * N_LOC + _l_of_i(pos_of)

    xT = np.zeros((N_CORES, IN_F, N_LOC), dtype=np.float32)
    degt = np.ones((N_CORES, P, N_TILES), dtype=np.float32)
    for k in range(N_CORES):
        nodes_k = order[k::N_CORES]
        i = np.arange(len(nodes_k))
        xT[k, :, i] = x[nodes_k]
        degt[k, i % P, i // P] = deg[nodes_k]

    # per-tile widths (same across cores; excludes self loop -> in-deg only)
    W = np.zeros(N_TILES, dtype=np.int64)
    indeg = deg - 1
    for t in range(N_TILES):
        lo, hi = t * P, (t + 1) * P
        sel = (pos_of >= lo) & (pos_of < hi)
        W[t] = indeg[sel].max() if sel.any() else 0

    # idx blobs: per core, per group: [W_g, TILE_GROUP, P] int32 table rows
    # (slot (w, tt, p) = w-th in-edge of dst node i=(g*TG+tt)*P+p), padded
    # with the core-local zero row.
    # emission order: width-major — for w in range(maxW): for t where W[t] > w.
    # Blob column r corresponds to the r-th emitted indirect DMA.
    e_core = core_of[dst]
    maxW = int(W.max())
    emit = [(w, t) for w in range(maxW) for t in range(N_TILES) if W[t] > w]
    idx_blobs = []
    for k in range(N_CORES):
        pad_row = k * N_LOC + _l_of_i(PAD_I)
        m = e_core == k
        s_k, d_k = src[m], dst[m]
        pos_d = pos_of[d_k]
        o = np.argsort(pos_d, kind="stable")
        s_k, pos_d = s_k[o], pos_d[o]
        bounds = np.searchsorted(pos_d, np.arange(n_real + 1))
        # slot table [P, N_TILES, maxW] filled with pad rows
        slot = np.full((P, N_TILES, maxW), pad_row, dtype=np.int32)
        for t in range(N_TILES):
            for p in range(P):
                i = t * P + p
                if i >= n_real:
                    continue
                lo, hi = bounds[i], bounds[i + 1]
                slot[p, t, : hi - lo] = trow_of[s_k[lo:hi]]
        blob = np.empty((P, len(emit)), dtype=np.int32)
        for r, (w, t) in enumerate(emit):
            blob[:, r] = slot[:, t, w]
        idx_blobs.append(blob)

    W2p = np.zeros((HID, HID2), dtype=np.float32)
    W2p[:, :N_CLS] = np.asarray(W2, dtype=np.float32)
    b1t = np.broadcast_to(np.asarray(b1, np.float32), (P, HID)).copy()
    b2t = np.zeros((P, HID2), dtype=np.float32)
    b2t[:, :N_CLS] = np.asarray(b2, np.float32)

    return dict(xT=xT, degt=degt, idx=np.stack(idx_blobs), W=W,
                W1=np.asarray(W1, np.float32), W2p=W2p, b1t=b1t, b2t=b2t,
                order=order)


_BUILD_CACHE = {}


def build(W):
    """Build + compile the SPMD bass program for group widths W (tuple)."""
    key = tuple(int(w) for w in W)
    if key in _BUILD_CACHE:
        return _BUILD_CACHE[key]

    from concourse import bass, mybir, bacc
    import concourse.tile as tile
    from concourse.masks import make_identity

    W = [int(w) for w in W]
    maxW = max(W)
    emit = [(w, t) for w in range(maxW) for t in range(N_TILES) if W[t] > w]
    IDX_ROWS = len(emit)

    nc = bacc.Bacc("TRN2", target_bir_lowering=False, debug=False,
                   num_devices=N_CORES)
    f32, i32 = mybir.dt.float32, mybir.dt.int32

    xT_in = nc.dram_tensor("xT", [IN_F, N_LOC], f32, kind="ExternalInput")
    degt_in = nc.dram_tensor("degt", [P, N_TILES], f32, kind="ExternalInput")
    idx_in = nc.dram_tensor("idx", [P, IDX_ROWS], i32, kind="ExternalInput")
    W1_in = nc.dram_tensor("W1", [IN_F, HID], f32, kind="ExternalInput")
    W2_in = nc.dram_tensor("W2p", [HID, HID2], f32, kind="ExternalInput")
    b1_in = nc.dram_tensor("b1t", [P, HID], f32, kind="ExternalInput")
    b2_in = nc.dram_tensor("b2t", [P, HID2], f32, kind="ExternalInput")
    out_ext = nc.dram_tensor("out", [N_LOC, N_CLS], f32, kind="ExternalOutput")

    shard1 = nc.dram_tensor("shard1", [N_LOC, HID], f32)
    table1 = nc.dram_tensor("table1", [TROWS, HID], f32, addr_space="Shared")
    shard2 = nc.dram_tensor("shard2", [N_LOC, HID2], f32)
    table2 = nc.dram_tensor("table2", [TROWS, HID2], f32, addr_space="Shared")

    with tile.TileContext(nc) as tc:
        with (
            tc.tile_pool(name="const", bufs=1) as cpool,
            tc.tile_pool(name="xts", bufs=2) as xpool,
            tc.tile_pool(name="idxp", bufs=1) as ipool,
            tc.tile_pool(name="work", bufs=1) as wpool,
            tc.tile_pool(name="psum", bufs=2, space="PSUM") as psum,
            tc.tile_pool(name="psum2", bufs=2, space="PSUM") as psum2,
        ):
            w1 = cpool.tile([P, 4 * HID], f32)  # 4 k-chunks side by side
            for kc in range(4):
                nc.sync.dma_start(out=w1[:, kc * HID:(kc + 1) * HID],
                                  in_=W1_in[kc * P:(kc + 1) * P, :])
            w2 = cpool.tile([HID, HID2], f32)
            nc.sync.dma_start(out=w2[:], in_=W2_in[:, :])
            b1 = cpool.tile([P, HID], f32)
            nc.sync.dma_start(out=b1[:], in_=b1_in[:, :])
            b2 = cpool.tile([P, HID2], f32)
            nc.sync.dma_start(out=b2[:], in_=b2_in[:, :])
            degt = cpool.tile([P, N_TILES], f32)
            nc.sync.dma_start(out=degt[:], in_=degt_in[:, :])
            dinv = cpool.tile([P, N_TILES], f32)
            nc.scalar.activation(dinv[:], degt[:],
                                 mybir.ActivationFunctionType.Sqrt)
            nc.vector.reciprocal(dinv[:], dinv[:])
            ident = cpool.tile([P, P], f32)
            make_identity(nc, ident[:])

            # whole idx blob resident in SBUF: [P, IDX_ROWS] int32
            idxs = ipool.tile([P, IDX_ROWS], i32)
            nc.sync.dma_start(out=idxs[:], in_=idx_in[:, :])

            hp = wpool.tile([P, N_TILES * HID], f32)   # h' rows (pre-scaled)
            # ---- Phase A: h' = dinv * (x @ W1), node tiles of 128 ----
            CH = TILE_GROUP * P  # 896 node-columns per chunk
            for g in range(N_GROUPS):
                xt = []
                for kc in range(4):
                    xtk = xpool.tile([P, CH], f32, tag=f"xt{kc}")
                    xt.append(xtk)
                for kc in range(4):
                    nc.sync.dma_start(
                        out=xt[kc][:],
                        in_=xT_in[kc * P:(kc + 1) * P, g * CH:(g + 1) * CH])
                for tt in range(TILE_GROUP):
                    t = g * TILE_GROUP + tt
                    ph = psum.tile([P, HID], f32, tag="ph")
                    for kc in range(4):
                        nc.tensor.matmul(
                            out=ph[:],
                            lhsT=xt[kc][:, tt * P:(tt + 1) * P],
                            rhs=w1[:, kc * HID:(kc + 1) * HID],
                            start=(kc == 0), stop=(kc == 3))
                    nc.vector.tensor_tensor(
                        out=hp[:, t * HID:(t + 1) * HID],
                        in0=ph[:],
                        in1=dinv[:, t:t + 1].to_broadcast([P, HID]),
                        op=mybir.AluOpType.mult)

            nc.sync.dma_start(
                out=shard1[:, :].rearrange("(p t) f -> p (t f)", p=P),
                in_=hp[:])
            nc.gpsimd.collective_compute(
                "AllGather", mybir.AluOpType.bypass,
                replica_groups=[list(range(N_CORES))],
                ins=[shard1[:, :]], outs=[table1[:, :]])

            # ---- Phase B: L1 aggregation via gather-accumulate ----
            agg1 = wpool.tile([P, N_TILES * HID], f32)
            # init agg = h' (self-loop contribution; rows already dinv-scaled)
            nc.vector.tensor_copy(out=agg1[:], in_=hp[:])

            for r, (w, t) in enumerate(emit):
                nc.gpsimd.indirect_dma_start(
                    out=agg1[:, t * HID:(t + 1) * HID],
                    out_offset=None,
                    in_=table1[:, :],
                    in_offset=bass.IndirectOffsetOnAxis(
                        ap=idxs[:, r:r + 1], axis=0),
                    compute_op=mybir.AluOpType.add)

            # h1 = relu(dinv*agg1 + b1)  (in place on agg1)
            for t in range(N_TILES):
                nc.vector.tensor_tensor(
                    out=agg1[:, t * HID:(t + 1) * HID],
                    in0=agg1[:, t * HID:(t + 1) * HID],
                    in1=dinv[:, t:t + 1].to_broadcast([P, HID]),
                    op=mybir.AluOpType.mult)
            a3 = agg1[:].rearrange("p (t f) -> p t f", f=HID)
            nc.vector.tensor_tensor(
                out=a3, in0=a3,
                in1=b1[:].rearrange("p (o f) -> p o f", o=1).to_broadcast(
                    [P, N_TILES, HID]),
                op=mybir.AluOpType.add)
            nc.scalar.activation(agg1[:], agg1[:],
                                 mybir.ActivationFunctionType.Relu)

            # ---- Phase C: h2' = dinv * (h1 @ W2p) ----
            h2p = wpool.tile([P, N_TILES * HID2], f32)
            for t in range(N_TILES):
                pt = psum.tile([HID, P], f32, tag="pt")
                nc.tensor.transpose(out=pt[:], in_=agg1[:, t * HID:(t + 1) * HID],
                                    identity=ident[:])
                h1T = wpool.tile([HID, P], f32, tag="h1T")
                nc.vector.tensor_copy(out=h1T[:], in_=pt[:])
                p2 = psum2.tile([P, HID2], f32, tag="p2")
                nc.tensor.matmul(out=p2[:], lhsT=h1T[:], rhs=w2[:],
                                 start=True, stop=True)
                nc.vector.tensor_tensor(
                    out=h2p[:, t * HID2:(t + 1) * HID2],
                    in0=p2[:],
                    in1=dinv[:, t:t + 1].to_broadcast([P, HID2]),
                    op=mybir.AluOpType.mult)

            nc.sync.dma_start(
                out=shard2[:, :].rearrange("(p t) f -> p (t f)", p=P),
                in_=h2p[:])
            nc.gpsimd.collective_compute(
                "AllGather", mybir.AluOpType.bypass,
                replica_groups=[list(range(N_CORES))],
                ins=[shard2[:, :]], outs=[table2[:, :]])

            # ---- Phase D: L2 aggregation ----
            agg2 = wpool.tile([P, N_TILES * HID2], f32)
            nc.vector.tensor_copy(out=agg2[:], in_=h2p[:])
            for r, (w, t) in enumerate(emit):
                nc.gpsimd.indirect_dma_start(
                    out=agg2[:, t * HID2:(t + 1) * HID2],
                    out_offset=None,
                    in_=table2[:, :],
                    in_offset=bass.IndirectOffsetOnAxis(
                        ap=idxs[:, r:r + 1], axis=0),
                    compute_op=mybir.AluOpType.add)

            # ---- Phase E: z = dinv*agg2 + b2; log_softmax over 7 cols ----
            for t in range(N_TILES):
                nc.vector.tensor_tensor(
                    out=agg2[:, t * HID2:(t + 1) * HID2],
                    in0=agg2[:, t * HID2:(t + 1) * HID2],
                    in1=dinv[:, t:t + 1].to_broadcast([P, HID2]),
                    op=mybir.AluOpType.mult)
            g3 = agg2[:].rearrange("p (t f) -> p t f", f=HID2)
            nc.vector.tensor_tensor(
                out=g3, in0=g3,
                in1=b2[:].rearrange("p (o f) -> p o f", o=1).to_broadcast(
                    [P, N_TILES, HID2]),
                op=mybir.AluOpType.add)

            z = agg2[:].rearrange("p (t f) -> p t f", f=HID2)
            z7 = z[:, :, 0:N_CLS]
            m = wpool.tile([P, N_TILES], f32)
            nc.vector.tensor_reduce(out=m[:], in_=z7, axis=mybir.AxisListType.X,
                                    op=mybir.AluOpType.max)
            zs = wpool.tile([P, N_TILES * N_CLS], f32)
            zs3 = zs[:].rearrange("p (t f) -> p t f", f=N_CLS)
            nc.vector.tensor_tensor(
                out=zs3, in0=z7,
                in1=m[:].rearrange("p (t o) -> p t o", o=1).to_broadcast(
                    [P, N_TILES, N_CLS]),
                op=mybir.AluOpType.subtract)
            e = wpool.tile([P, N_TILES * N_CLS], f32)
            nc.scalar.activation(e[:], zs[:], mybir.ActivationFunctionType.Exp)
            s = wpool.tile([P, N_TILES], f32)
            nc.vector.tensor_reduce(
                out=s[:], in_=e[:].rearrange("p (t f) -> p t f", f=N_CLS),
                axis=mybir.AxisListType.X, op=mybir.AluOpType.add)
            ls = wpool.tile([P, N_TILES], f32)
            nc.scalar.activation(ls[:], s[:], mybir.ActivationFunctionType.Ln)
            nc.vector.tensor_tensor(
                out=zs3, in0=zs3,
                in1=ls[:].rearrange("p (t o) -> p t o", o=1).to_broadcast(
                    [P, N_TILES, N_CLS]),
                op=mybir.AluOpType.subtract)

            nc.sync.dma_start(
                out=out_ext[:, :].rearrange("(p t) f -> p (t f)", p=P),
                in_=zs[:])
    nc.compile()
    _BUILD_CACHE[key] = nc
    return nc


def kernel(x, W1, b1, W2, b2, edge_index):
    from concourse.bass_utils import run_bass_kernel_spmd

    prep = preprocess(x, W1, b1, W2, b2, edge_index)
    nc = build(prep["W"])

    in_maps = []
    for k in range(N_CORES):
        in_maps.append({
            "xT": prep["xT"][k],
            "degt": prep["degt"][k],
            "idx": prep["idx"][k],
            "W1": prep["W1"],
            "W2p": prep["W2p"],
            "b1t": prep["b1t"],
            "b2t": prep["b2t"],
        })

    global LAST_EXEC_NS
    if os.environ.get("GCN_SIM", "") == "1":
        from concourse import bass_interp
        sim = bass_interp.MultiCoreSim(nc, N_CORES)
        for k in range(N_CORES):
            for name, arr in in_maps[k].items():
                sim.cores[k].tensor(name)[:] = arr
        sim.simulate()
        class _R: pass
        res = _R()
        res.results = [{"out": np.array(sim.cores[k].tensor("out"))}
                       for k in range(N_CORES)]
        LAST_EXEC_NS = int(sim.global_time)
    else:
        trace = os.environ.get("GCN_TRACE", "") == "1"
        try:
            res = run_bass_kernel_spmd(nc, in_maps,
                                       core_ids=list(range(N_CORES)),
                                       trace=trace)
        except ModuleNotFoundError:
            res = run_bass_kernel_spmd(nc, in_maps,
                                       core_ids=list(range(N_CORES)))
        LAST_EXEC_NS = res.exec_time_ns

    order = prep["order"]
    out = np.zeros((N_NODES, N_CLS), dtype=np.float32)
    i = np.arange(N_NODES // N_CORES)
    li = _l_of_i(i)
    for k in range(N_CORES):
        out[order[k::N_CORES]] = res.results[k]["out"][li]
    return out


LAST_EXEC_NS = None
